# revision 8
# baseline (speedup 1.0000x reference)
"""Trainium2 Bass kernel for nn_AMN_QP: MLP head + 30 QP gradient-descent
iterations with momentum, data-parallel over 8 NeuronCores.

Math (per batch row):
    V0 = relu(x @ W1 + b1) @ W2 + b2
    repeat n_iteration times:
        dV = 2/256 (V Sᵀ) S + 2/128 relu(V Pinᵀ - Vin) Pin + 2/512 min(V, 0)
        diff = 0.9 diff - 0.01 dV
        V += diff

Kernel formulation (everything transposed: [feature, batch] in SBUF,
batch tiles of 512 as the matmul free dim):
    g := -0.01*dVᵀ accumulated in PSUM by TensorE from
         LG = -0.01*2/256*(SᵀS)      (K=512)      @ Vᵀ
         LP = -0.01*2/128*Pin        (K=128)      @ reluᵀ
         γI = +0.01*2/512*I          (K=128)      @ relu(-V)ᵀ   [min(V,0) = -relu(-V)]
    Bᵀ  := Pin@Vᵀ - Vin  accumulated in PSUM from [Pinᵀ; -I] @ [Vᵀ; Vinᵀ]
    DVE: reluᵀ = max(Bᵀ,0);  D = 0.9*D + g  (one fused scalar_tensor_tensor)
    ACT: R4 = relu(-Vᵀ)
    GpSimd: V += D
All matmul operands are float32r (1 cycle/row on TensorE, ~2^-13 rounding).
"""

import numpy as np

import concourse.bass as bass
import concourse.mybir as mybir
import concourse.tile as tile
from concourse import bacc
from concourse.bass_utils import run_bass_kernel_spmd

P = 128
N_CORES = 8
B_FULL = 32768
D_IN = 128
H = 1024
N_FLUX = 512
N_IN = 128
N_MET = 256
LR = 0.01
DECAY = 0.9

BT = 512          # batch tile (matmul free dim)
MC = N_FLUX // P  # 4 flux chunks
HC = H // P       # 8 hidden chunks

F32 = mybir.dt.float32
F32R = mybir.dt.float32r
ALU = mybir.AluOpType
ACTF = mybir.ActivationFunctionType


def _build(n_iter: int, n_tiles: int, use_for_i: bool = True):
    """One NeuronCore program for a shard of n_tiles*512 batch rows."""
    nc = bacc.Bacc()
    b_shard = n_tiles * BT
    JT = b_shard // P  # 128-row blocks in shard

    x_d = nc.declare_dram_parameter("x", [b_shard, D_IN], F32, isOutput=False)
    vin_d = nc.declare_dram_parameter("vin", [b_shard, N_IN], F32, isOutput=False)
    w1_d = nc.declare_dram_parameter("w1", [D_IN, H], F32R, isOutput=False)
    w2_d = nc.declare_dram_parameter("w2", [H, N_FLUX], F32R, isOutput=False)
    b1_d = nc.declare_dram_parameter("b1", [H], F32, isOutput=False)
    b2_d = nc.declare_dram_parameter("b2", [N_FLUX], F32, isOutput=False)
    lg_d = nc.declare_dram_parameter("lg", [N_FLUX, N_FLUX], F32R, isOutput=False)
    lp_d = nc.declare_dram_parameter("lp", [N_IN, N_FLUX], F32R, isOutput=False)
    # [Pinᵀ; -I]: 5*128 x 128
    pt_d = nc.declare_dram_parameter("pt", [N_FLUX + N_IN, N_IN], F32R, isOutput=False)
    idn_d = nc.declare_dram_parameter("idn", [P, P], F32, isOutput=False)
    idnr_d = nc.declare_dram_parameter("idnr", [P, P], F32R, isOutput=False)
    out_d = nc.declare_dram_parameter("out", [b_shard, N_FLUX], F32, isOutput=True)

    with tile.TileContext(nc) as tc:
        with (
            tc.tile_pool(name="state", bufs=1) as st,
            tc.tile_pool(name="scratch", bufs=2) as sc,
            tc.tile_pool(name="h1p", bufs=1) as h1p,
            tc.tile_pool(name="psA", bufs=2, space="PSUM") as psA,
            tc.tile_pool(name="psB", bufs=5, space="PSUM") as psB,
            tc.tile_pool(name="psT", bufs=1, space="PSUM") as psT,
        ):
            # ---- persistent SBUF state ----
            v_sb = st.tile([P, n_tiles, MC, BT], F32R)      # Vᵀ
            d_sb = st.tile([P, n_tiles, MC, BT], mybir.dt.bfloat16)  # diffᵀ
            vint = st.tile([P, n_tiles, BT], F32R)          # Vinᵀ
            w1 = st.tile([P, HC, P], F32R)
            w2 = st.tile([P, HC, MC, P], F32R)
            lg = st.tile([P, MC, MC, P], F32R)
            lp = st.tile([P, MC, P], F32R)
            pt = st.tile([P, MC + 1, P], F32R)
            idn = st.tile([P, P], F32)
            idnr = st.tile([P, P], F32R)
            b1 = st.tile([P, HC], F32)
            b2 = st.tile([P, MC], F32)

            nc.sync.dma_start(w1[:], w1_d.rearrange("p (m q) -> p m q", q=P))
            nc.sync.dma_start(w2[:], w2_d.rearrange("(k p) (m q) -> p k m q", p=P, q=P))
            nc.sync.dma_start(lg[:], lg_d.rearrange("(k p) (m q) -> p k m q", p=P, q=P))
            nc.sync.dma_start(lp[:], lp_d.rearrange("p (m q) -> p m q", q=P))
            nc.sync.dma_start(pt[:], pt_d.rearrange("(k p) q -> p k q", p=P))
            nc.sync.dma_start(idn[:], idn_d[:])
            nc.sync.dma_start(idnr[:], idnr_d[:])
            nc.sync.dma_start(b1[:], b1_d.rearrange("(m p) -> p m", p=P))
            nc.sync.dma_start(b2[:], b2_d.rearrange("(m p) -> p m", p=P))

            nc.gpsimd.memset(d_sb[:], 0.0)

            # ---- per-tile input transposes + MLP head -> Vᵀ ----
            x4 = x_d.rearrange("(t j p) f -> t p j f", p=P, j=4)
            vin4 = vin_d.rearrange("(t j p) f -> t p j f", p=P, j=4)
            for t in range(n_tiles):
                x_raw = sc.tile([P, 4, D_IN], F32, tag="xraw")
                vin_raw = sc.tile([P, 4, N_IN], F32, tag="vinraw")
                nc.sync.dma_start(x_raw[:], x4[t])
                nc.sync.dma_start(vin_raw[:], vin4[t])
                xt = sc.tile([P, 4, P], F32R, tag="xt")
                for j in range(4):
                    pstp = psT.tile([P, P], F32, tag="pst")
                    nc.tensor.transpose(pstp[:], x_raw[:, j], idn[:])
                    nc.vector.tensor_copy(xt[:, j], pstp[:])
                    pstv = psT.tile([P, P], F32, tag="pst")
                    nc.tensor.transpose(pstv[:], vin_raw[:, j], idn[:])
                    nc.vector.tensor_copy(vint[:, t, bass.ts(j, P)], pstv[:])

                h1 = h1p.tile([P, HC, BT], F32R, tag="h1")
                xt2 = xt[:].rearrange("p j q -> p (j q)")
                for m in range(HC):
                    ps = psA.tile([P, BT], F32, tag="psA")
                    nc.tensor.matmul(ps[:], w1[:, m], xt2, start=True, stop=True)
                    nc.scalar.activation(
                        h1[:, m], ps[:], ACTF.Relu, bias=b1[:, m : m + 1]
                    )
                for m in range(MC):
                    ps = psB.tile([P, BT], F32, tag="psB")
                    for k in range(HC):
                        nc.tensor.matmul(
                            ps[:], w2[:, k, m], h1[:, k],
                            start=(k == 0), stop=(k == HC - 1),
                        )
                    nc.vector.tensor_scalar_add(v_sb[:, t, m], ps[:], b2[:, m : m + 1])

            # ---- QP iterations ----
            def body(_i=None):
                for t in range(n_tiles):
                    r4 = sc.tile([P, MC, BT], mybir.dt.bfloat16, tag="r4b")
                    for m in range(MC):
                        # gamma*relu(-V) == relu(-gamma*V); rides into D on DVE
                        nc.scalar.activation(
                            r4[:, m], v_sb[:, t, m], ACTF.Relu,
                            scale=-(LR * 2.0 / N_FLUX),
                        )
                    bt_ps = psA.tile([P, BT], F32, tag="psA")
                    for k in range(MC):
                        nc.tensor.matmul(
                            bt_ps[:], pt[:, k], v_sb[:, t, k],
                            start=(k == 0), stop=False,
                        )
                    nc.tensor.matmul(
                        bt_ps[:], pt[:, MC], vint[:, t], start=False, stop=True
                    )
                    relu = sc.tile([P, BT], F32R, tag="relu")
                    nc.scalar.activation(relu[:], bt_ps[:], ACTF.Relu)
                    for m in range(MC):
                        g_ps = psB.tile([P, BT], F32, tag="psB")
                        for k in range(MC):
                            nc.tensor.matmul(
                                g_ps[:], lg[:, k, m], v_sb[:, t, k],
                                start=(k == 0), stop=False,
                            )
                        nc.tensor.matmul(
                            g_ps[:], lp[:, m], relu[:], start=False, stop=True
                        )
                        nc.vector.scalar_tensor_tensor(
                            d_sb[:, t, m], d_sb[:, t, m], DECAY, g_ps[:],
                            op0=ALU.mult, op1=ALU.add,
                        )
                        nc.vector.tensor_add(
                            out=d_sb[:, t, m], in0=d_sb[:, t, m], in1=r4[:, m]
                        )
                    # V updates strictly after every g-group read V (Jacobi)
                    for m in range(MC):
                        nc.gpsimd.tensor_add(
                            out=v_sb[:, t, m], in0=v_sb[:, t, m], in1=d_sb[:, t, m]
                        )

            if use_for_i and n_iter > 1:
                with tc.For_i(
                    0, n_iter, 1, hint_engines=(mybir.EngineType.PE,)
                ) as _i:
                    body(_i)
            else:
                for _ in range(n_iter):
                    body()

            # ---- transpose back + store ----
            out3 = out_d.rearrange("(t j p) f -> t p j f", p=P, j=4)
            for t in range(n_tiles):
                vo = sc.tile([P, 4, N_FLUX], F32, tag="r4b")
                for m in range(MC):
                    for j in range(4):
                        pso = psT.tile([P, P], F32R, tag="pst")
                        nc.tensor.transpose(
                            pso[:], v_sb[:, t, m, bass.ts(j, P)],
                            idnr[:],
                        )
                        nc.vector.tensor_copy(vo[:, j, bass.ts(m, P)], pso[:])
                nc.sync.dma_start(out3[t], vo[:])
    nc.compile()
    return nc


def _host_weights(W1, b1, W2, b2, S, Pin):
    S64 = S.astype(np.float64)
    G = S64.T @ S64
    LG = (-LR * 2.0 / N_MET * G).astype(np.float32)
    LP = (-LR * 2.0 / N_IN * Pin.astype(np.float64)).astype(np.float32)
    PT = np.concatenate(
        [Pin.astype(np.float32).T, -np.eye(N_IN, dtype=np.float32)], axis=0
    )
    IDN = np.eye(P, dtype=np.float32)
    return {
        "w1": np.ascontiguousarray(W1, dtype=np.float32),
        "w2": np.ascontiguousarray(W2, dtype=np.float32),
        "b1": np.ascontiguousarray(b1, dtype=np.float32),
        "b2": np.ascontiguousarray(b2, dtype=np.float32),
        "lg": np.ascontiguousarray(LG),
        "lp": np.ascontiguousarray(LP),
        "pt": np.ascontiguousarray(PT),
        "idn": np.ascontiguousarray(IDN),
        "idnr": np.ascontiguousarray(IDN),
    }


def run_sharded(inputs, n_iter, n_tiles_per_core=8, use_for_i=True, trace=False,
                nc=None):
    """Shard batch across 8 cores, run, gather. Returns (out, bass_results)."""
    x = np.asarray(inputs["input"], dtype=np.float32)
    vin = np.asarray(inputs["Vin"], dtype=np.float32)
    b = x.shape[0]
    b_shard = n_tiles_per_core * BT
    assert b == N_CORES * b_shard, (b, b_shard)

    wts = _host_weights(
        inputs["W1"], inputs["b1"], inputs["W2"], inputs["b2"],
        inputs["S"], inputs["Pin"],
    )
    if nc is None:
        nc = _build(n_iter, n_tiles_per_core, use_for_i)
    in_maps = []
    for c in range(N_CORES):
        sl = slice(c * b_shard, (c + 1) * b_shard)
        in_maps.append({"x": x[sl], "vin": vin[sl], **wts})
    r = run_bass_kernel_spmd(nc, in_maps, list(range(N_CORES)), trace=trace)
    out = np.concatenate([r.results[c]["out"] for c in range(N_CORES)], axis=0)
    return out, r


def kernel(**inputs) -> np.ndarray:
    n_iter = int(inputs["n_iteration"])
    out, _ = run_sharded(inputs, n_iter)
    return out.astype(np.float32)


# revision 9
# speedup vs baseline: 1.0367x; 1.0367x over previous
"""Trainium2 Bass kernel for nn_AMN_QP: MLP head + 30 QP gradient-descent
iterations with momentum, data-parallel over 8 NeuronCores.

Math (per batch row):
    V0 = relu(x @ W1 + b1) @ W2 + b2
    repeat n_iteration times:
        dV = 2/256 (V Sᵀ) S + 2/128 relu(V Pinᵀ - Vin) Pin + 2/512 min(V, 0)
        diff = 0.9 diff - 0.01 dV
        V += diff

Kernel formulation (everything transposed: [feature, batch] in SBUF,
batch tiles of 512 as the matmul free dim):
    g := -0.01*dVᵀ accumulated in PSUM by TensorE from
         LG = -0.01*2/256*(SᵀS)      (K=512)      @ Vᵀ
         LP = -0.01*2/128*Pin        (K=128)      @ reluᵀ
         γI = +0.01*2/512*I          (K=128)      @ relu(-V)ᵀ   [min(V,0) = -relu(-V)]
    Bᵀ  := Pin@Vᵀ - Vin  accumulated in PSUM from [Pinᵀ; -I] @ [Vᵀ; Vinᵀ]
    DVE: reluᵀ = max(Bᵀ,0);  D = 0.9*D + g  (one fused scalar_tensor_tensor)
    ACT: R4 = relu(-Vᵀ)
    GpSimd: V += D
All matmul operands are float32r (1 cycle/row on TensorE, ~2^-13 rounding).
"""

import numpy as np

import concourse.bass as bass
import concourse.mybir as mybir
import concourse.tile as tile
from concourse import bacc
from concourse.bass_utils import run_bass_kernel_spmd

P = 128
N_CORES = 8
B_FULL = 32768
D_IN = 128
H = 1024
N_FLUX = 512
N_IN = 128
N_MET = 256
LR = 0.01
DECAY = 0.9

BT = 512          # batch tile (matmul free dim)
MC = N_FLUX // P  # 4 flux chunks
HC = H // P       # 8 hidden chunks

F32 = mybir.dt.float32
F32R = mybir.dt.float32r
ALU = mybir.AluOpType
ACTF = mybir.ActivationFunctionType


def _build(n_iter: int, n_tiles: int, use_for_i: bool = True):
    """One NeuronCore program for a shard of n_tiles*512 batch rows."""
    nc = bacc.Bacc()
    b_shard = n_tiles * BT
    JT = b_shard // P  # 128-row blocks in shard

    x_d = nc.declare_dram_parameter("x", [b_shard, D_IN], F32, isOutput=False)
    vin_d = nc.declare_dram_parameter("vin", [b_shard, N_IN], F32, isOutput=False)
    w1_d = nc.declare_dram_parameter("w1", [D_IN, H], F32R, isOutput=False)
    w2_d = nc.declare_dram_parameter("w2", [H, N_FLUX], F32R, isOutput=False)
    b1_d = nc.declare_dram_parameter("b1", [H], F32, isOutput=False)
    b2_d = nc.declare_dram_parameter("b2", [N_FLUX], F32, isOutput=False)
    lg_d = nc.declare_dram_parameter("lg", [N_FLUX, N_FLUX], F32R, isOutput=False)
    lp_d = nc.declare_dram_parameter("lp", [N_IN, N_FLUX], F32R, isOutput=False)
    # [Pinᵀ; -I]: 5*128 x 128
    pt_d = nc.declare_dram_parameter("pt", [N_FLUX + N_IN, N_IN], F32R, isOutput=False)
    idn_d = nc.declare_dram_parameter("idn", [P, P], F32, isOutput=False)
    idnr_d = nc.declare_dram_parameter("idnr", [P, P], F32R, isOutput=False)
    out_d = nc.declare_dram_parameter("out", [b_shard, N_FLUX], F32, isOutput=True)

    with tile.TileContext(nc) as tc:
        with (
            tc.tile_pool(name="state", bufs=1) as st,
            tc.tile_pool(name="scratch", bufs=2) as sc,
            tc.tile_pool(name="h1p", bufs=1) as h1p,
            tc.tile_pool(name="psA", bufs=2, space="PSUM") as psA,
            tc.tile_pool(name="psB", bufs=5, space="PSUM") as psB,
            tc.tile_pool(name="psT", bufs=1, space="PSUM") as psT,
        ):
            # ---- persistent SBUF state ----
            v_sb = st.tile([P, n_tiles, MC, BT], F32R)      # Vᵀ
            d_sb = st.tile([P, n_tiles, MC, BT], mybir.dt.bfloat16)  # diffᵀ
            vint = st.tile([P, n_tiles, BT], F32R)          # Vinᵀ
            w1 = st.tile([P, HC, P], F32R)
            w2 = st.tile([P, HC, MC, P], F32R)
            lg = st.tile([P, MC, MC, P], F32R)
            lp = st.tile([P, MC, P], F32R)
            pt = st.tile([P, MC + 1, P], F32R)
            idn = st.tile([P, P], F32)
            idnr = st.tile([P, P], F32R)
            b1 = st.tile([P, HC], F32)
            b2 = st.tile([P, MC], F32)

            nc.sync.dma_start(w1[:], w1_d.rearrange("p (m q) -> p m q", q=P))
            nc.sync.dma_start(w2[:], w2_d.rearrange("(k p) (m q) -> p k m q", p=P, q=P))
            nc.sync.dma_start(lg[:], lg_d.rearrange("(k p) (m q) -> p k m q", p=P, q=P))
            nc.sync.dma_start(lp[:], lp_d.rearrange("p (m q) -> p m q", q=P))
            nc.sync.dma_start(pt[:], pt_d.rearrange("(k p) q -> p k q", p=P))
            nc.sync.dma_start(idn[:], idn_d[:])
            nc.sync.dma_start(idnr[:], idnr_d[:])
            nc.sync.dma_start(b1[:], b1_d.rearrange("(m p) -> p m", p=P))
            nc.sync.dma_start(b2[:], b2_d.rearrange("(m p) -> p m", p=P))

            nc.gpsimd.memset(d_sb[:], 0.0)

            # ---- per-tile input transposes + MLP head -> Vᵀ ----
            x4 = x_d.rearrange("(t j p) f -> t p j f", p=P, j=4)
            vin4 = vin_d.rearrange("(t j p) f -> t p j f", p=P, j=4)
            for t in range(n_tiles):
                x_raw = sc.tile([P, 4, D_IN], F32, tag="xraw")
                vin_raw = sc.tile([P, 4, N_IN], F32, tag="vinraw")
                nc.sync.dma_start(x_raw[:], x4[t])
                nc.sync.dma_start(vin_raw[:], vin4[t])
                xt = sc.tile([P, 4, P], F32R, tag="xt")
                for j in range(4):
                    pstp = psT.tile([P, P], F32, tag="pst")
                    nc.tensor.transpose(pstp[:], x_raw[:, j], idn[:])
                    nc.vector.tensor_copy(xt[:, j], pstp[:])
                    pstv = psT.tile([P, P], F32, tag="pst")
                    nc.tensor.transpose(pstv[:], vin_raw[:, j], idn[:])
                    nc.vector.tensor_copy(vint[:, t, bass.ts(j, P)], pstv[:])

                h1 = h1p.tile([P, HC, BT], F32R, tag="h1")
                xt2 = xt[:].rearrange("p j q -> p (j q)")
                for m in range(HC):
                    ps = psA.tile([P, BT], F32, tag="psA")
                    nc.tensor.matmul(ps[:], w1[:, m], xt2, start=True, stop=True)
                    nc.scalar.activation(
                        h1[:, m], ps[:], ACTF.Relu, bias=b1[:, m : m + 1]
                    )
                for m in range(MC):
                    ps = psB.tile([P, BT], F32, tag="psB")
                    for k in range(HC):
                        nc.tensor.matmul(
                            ps[:], w2[:, k, m], h1[:, k],
                            start=(k == 0), stop=(k == HC - 1),
                        )
                    nc.vector.tensor_scalar_add(v_sb[:, t, m], ps[:], b2[:, m : m + 1])

            # ---- QP iterations ----
            def body(_i=None):
                for t in range(n_tiles):
                    r4 = sc.tile([P, MC, BT], mybir.dt.bfloat16, tag="r4b")
                    for m in range(MC):
                        # gamma*relu(-V) == relu(-gamma*V); rides into D on DVE
                        nc.scalar.activation(
                            r4[:, m], v_sb[:, t, m], ACTF.Relu,
                            scale=-(LR * 2.0 / N_FLUX),
                        )
                    # early half of the D update runs under the matmuls
                    for m in range(MC):
                        nc.vector.scalar_tensor_tensor(
                            d_sb[:, t, m], d_sb[:, t, m], DECAY, r4[:, m],
                            op0=ALU.mult, op1=ALU.add,
                        )
                    bt_ps = psA.tile([P, BT], F32, tag="psA")
                    for k in range(MC):
                        nc.tensor.matmul(
                            bt_ps[:], pt[:, k], v_sb[:, t, k],
                            start=(k == 0), stop=False,
                        )
                    nc.tensor.matmul(
                        bt_ps[:], pt[:, MC], vint[:, t], start=False, stop=True
                    )
                    relu = sc.tile([P, BT], F32R, tag="relu")
                    nc.scalar.activation(relu[:], bt_ps[:], ACTF.Relu)
                    for m in range(MC):
                        g_ps = psB.tile([P, BT], F32, tag="psB")
                        for k in range(MC):
                            nc.tensor.matmul(
                                g_ps[:], lg[:, k, m], v_sb[:, t, k],
                                start=(k == 0), stop=False,
                            )
                        nc.tensor.matmul(
                            g_ps[:], lp[:, m], relu[:], start=False, stop=True
                        )
                        nc.vector.tensor_add(
                            out=d_sb[:, t, m], in0=d_sb[:, t, m], in1=g_ps[:]
                        )
                    # V updates strictly after every g-group read V (Jacobi)
                    for m in range(MC):
                        if t == n_tiles - 1:
                            nc.vector.tensor_add(
                                out=v_sb[:, t, m], in0=v_sb[:, t, m],
                                in1=d_sb[:, t, m],
                            )
                        else:
                            nc.gpsimd.tensor_add(
                                out=v_sb[:, t, m], in0=v_sb[:, t, m],
                                in1=d_sb[:, t, m],
                            )

            if use_for_i and n_iter > 1:
                with tc.For_i(0, n_iter, 1) as _i:
                    body(_i)
            else:
                for _ in range(n_iter):
                    body()

            # ---- transpose back + store ----
            out3 = out_d.rearrange("(t j p) f -> t p j f", p=P, j=4)
            for t in range(n_tiles):
                vo = sc.tile([P, 4, N_FLUX], F32, tag="r4b")
                for m in range(MC):
                    for j in range(4):
                        pso = psT.tile([P, P], F32R, tag="pst")
                        nc.tensor.transpose(
                            pso[:], v_sb[:, t, m, bass.ts(j, P)],
                            idnr[:],
                        )
                        nc.vector.tensor_copy(vo[:, j, bass.ts(m, P)], pso[:])
                nc.sync.dma_start(out3[t], vo[:])
    nc.compile()
    return nc


def _host_weights(W1, b1, W2, b2, S, Pin):
    S64 = S.astype(np.float64)
    G = S64.T @ S64
    LG = (-LR * 2.0 / N_MET * G).astype(np.float32)
    LP = (-LR * 2.0 / N_IN * Pin.astype(np.float64)).astype(np.float32)
    PT = np.concatenate(
        [Pin.astype(np.float32).T, -np.eye(N_IN, dtype=np.float32)], axis=0
    )
    IDN = np.eye(P, dtype=np.float32)
    return {
        "w1": np.ascontiguousarray(W1, dtype=np.float32),
        "w2": np.ascontiguousarray(W2, dtype=np.float32),
        "b1": np.ascontiguousarray(b1, dtype=np.float32),
        "b2": np.ascontiguousarray(b2, dtype=np.float32),
        "lg": np.ascontiguousarray(LG),
        "lp": np.ascontiguousarray(LP),
        "pt": np.ascontiguousarray(PT),
        "idn": np.ascontiguousarray(IDN),
        "idnr": np.ascontiguousarray(IDN),
    }


def run_sharded(inputs, n_iter, n_tiles_per_core=8, use_for_i=True, trace=False,
                nc=None):
    """Shard batch across 8 cores, run, gather. Returns (out, bass_results)."""
    x = np.asarray(inputs["input"], dtype=np.float32)
    vin = np.asarray(inputs["Vin"], dtype=np.float32)
    b = x.shape[0]
    b_shard = n_tiles_per_core * BT
    assert b == N_CORES * b_shard, (b, b_shard)

    wts = _host_weights(
        inputs["W1"], inputs["b1"], inputs["W2"], inputs["b2"],
        inputs["S"], inputs["Pin"],
    )
    if nc is None:
        nc = _build(n_iter, n_tiles_per_core, use_for_i)
    in_maps = []
    for c in range(N_CORES):
        sl = slice(c * b_shard, (c + 1) * b_shard)
        in_maps.append({"x": x[sl], "vin": vin[sl], **wts})
    r = run_bass_kernel_spmd(nc, in_maps, list(range(N_CORES)), trace=trace)
    out = np.concatenate([r.results[c]["out"] for c in range(N_CORES)], axis=0)
    return out, r


def kernel(**inputs) -> np.ndarray:
    n_iter = int(inputs["n_iteration"])
    out, _ = run_sharded(inputs, n_iter)
    return out.astype(np.float32)
